# revision 11
# baseline (speedup 1.0000x reference)
"""Cross-view attention Trainium2 kernel.

Reference computation (per sample b):
    q = Wq @ x1 + bq            (D=64, N)      x1 = view1[b] as (C, N)
    k = Wk @ x2 + bk            (D, N)
    v = Wv @ x2 + bv            (C, N)
    S = q^T k                   (N, N)
    P = softmax(S, axis=-1)
    out = v @ P^T               (C, N)
    y = gamma * out + x1

Sharding: data-parallel over batch B=8 across the 8 NeuronCores (one
sample per core), no collectives.

Device algorithm (per core):
  - All matmuls in float32r (TF32-like, ~1.5e-4 rel err, full PE rate).
  - Projections computed directly in the layouts the attention needs:
      qT, kT as (D=64 partitions, N free), vT as (m partitions, C free).
  - Attention computed transposed: S^T tiles (m=128 partitions, n=512
    free) = kT_tile^T @ qT; exp on ScalarE (no max subtraction needed:
    logits are bounded ~+-50, exp stays in fp32 range); P^T tiles feed
    out[c,n] += vT^T @ expS^T accumulated over all m in PSUM, and the
    softmax denominator l[n] comes from a ones-column matmul accumulated
    alongside.  Final: out = (acc * (gamma/l)) + view1, streamed to HBM.
"""

import sys

if "/opt/trn_rl_repo" not in sys.path:
    sys.path.insert(0, "/opt/trn_rl_repo")

import numpy as np

B, C, H, W = 8, 512, 64, 64
D = C // 8            # 64
N = H * W             # 4096
CC = C // 128         # 4 chunks of the channel dim
NCORES = 8

_compiled = {}


def _build(n=N, repeat=1, nwin=512):
    from contextlib import ExitStack

    import concourse.mybir as mybir
    import concourse.tile as tile
    from concourse import bacc

    dt = mybir.dt
    f32, f32r = dt.float32, dt.float32r
    AF = mybir.ActivationFunctionType
    Alu = mybir.AluOpType

    nwin = min(nwin, n)
    nch = n // nwin       # output n-chunks
    mt = n // 128         # m tiles (key/value rows per tile)

    nc = bacc.Bacc("TRN2", target_bir_lowering=False, debug=False)
    v1 = nc.dram_tensor("v1", [C, n], f32, kind="ExternalInput").ap()
    v2 = nc.dram_tensor("v2", [C, n], f32, kind="ExternalInput").ap()
    wqT = nc.dram_tensor("wqT", [C, D], f32, kind="ExternalInput").ap()
    wkT = nc.dram_tensor("wkT", [C, D], f32, kind="ExternalInput").ap()
    wvT = nc.dram_tensor("wvT", [C, C], f32, kind="ExternalInput").ap()
    bq = nc.dram_tensor("bq", [1, D], f32, kind="ExternalInput").ap()
    bk = nc.dram_tensor("bk", [1, D], f32, kind="ExternalInput").ap()
    bv = nc.dram_tensor("bv", [1, C], f32, kind="ExternalInput").ap()
    gam = nc.dram_tensor("gam", [1, 1], f32, kind="ExternalInput").ap()
    out = nc.dram_tensor("out", [C, n], f32, kind="ExternalOutput").ap()

    v1p = v1.rearrange("(cc p) n -> p cc n", p=128)
    v2p = v2.rearrange("(cc p) n -> p cc n", p=128)
    outp = out.rearrange("(cc p) n -> p cc n", p=128)

    with tile.TileContext(nc) as tc, ExitStack() as top:
        consts = top.enter_context(tc.tile_pool(name="consts", bufs=1))

        # ---- constants: weights/biases rounded to f32r, ones vectors ----
        wq_s = consts.tile([128, CC, D], f32r, tag="wq")
        wk_s = consts.tile([128, CC, D], f32r, tag="wk")
        wv_s = consts.tile([128, CC, C], f32r, tag="wv")
        bq_s = consts.tile([1, D], f32r, tag="bq")
        bk_s = consts.tile([1, D], f32r, tag="bk")
        bv_s = consts.tile([1, C], f32r, tag="bv")
        gam_s = consts.tile([1, 1], f32, tag="gam")
        ones_row = consts.tile([1, nwin], f32r, tag="ones_row")  # K=1 rhs
        ones_col = consts.tile([128, 1], f32r, tag="ones_col")  # K=128, M=1 lhsT
        ones_p = consts.tile([1, 128], f32r, tag="ones_p")  # K=1, M=128 lhsT

        with ExitStack() as p0:
            wstp = p0.enter_context(tc.tile_pool(name="wst", bufs=1))
            stage_w = wstp.tile([128, CC, C], f32, tag="stage_w")
            nc.sync.dma_start(stage_w[:, :, :D], wqT.rearrange("(cc p) d -> p cc d", p=128))
            nc.vector.tensor_copy(wq_s[:], stage_w[:, :, :D])
            nc.sync.dma_start(stage_w[:, :, D : 2 * D], wkT.rearrange("(cc p) d -> p cc d", p=128))
            nc.vector.tensor_copy(wk_s[:], stage_w[:, :, D : 2 * D])
            nc.sync.dma_start(stage_w[:], wvT.rearrange("(cc p) c -> p cc c", p=128))
            nc.vector.tensor_copy(wv_s[:], stage_w[:])

            stage_b = wstp.tile([1, 2 * D + C + 1], f32, tag="stage_b")
            nc.sync.dma_start(stage_b[:, :D], bq[:])
            nc.sync.dma_start(stage_b[:, D : 2 * D], bk[:])
            nc.sync.dma_start(stage_b[:, 2 * D : 2 * D + C], bv[:])
            nc.sync.dma_start(stage_b[:, 2 * D + C :], gam[:])
            nc.vector.tensor_copy(bq_s[:], stage_b[:, :D])
            nc.vector.tensor_copy(bk_s[:], stage_b[:, D : 2 * D])
            nc.vector.tensor_copy(bv_s[:], stage_b[:, 2 * D : 2 * D + C])
            nc.vector.tensor_copy(gam_s[:], stage_b[:, 2 * D + C :])

            ones_f32 = wstp.tile([128, nwin], f32, tag="ones_f32")
            nc.vector.memset(ones_f32[:], 1.0)
            nc.vector.tensor_copy(ones_row[:], ones_f32[:1, :])
            nc.vector.tensor_copy(ones_col[:], ones_f32[:, :1])
            nc.vector.tensor_copy(ones_p[:], ones_f32[:1, :128])

        def emit_rep(rep):
            with ExitStack() as rctx:
                per = rctx.enter_context(
                    tc.tile_pool(name=f"persist{rep}", bufs=1)
                )
                qT_s = per.tile([64, n], f32r, tag="qT")
                kT_s = per.tile([64, n], f32r, tag="kT")
                vT_s = per.tile([128, mt, C], f32r, tag="vT")

                # ================= phase 1: projections =================
                with ExitStack() as p1:
                    xst = p1.enter_context(tc.tile_pool(name=f"xst{rep}", bufs=3))
                    xrp = p1.enter_context(tc.tile_pool(name=f"xrp{rep}", bufs=3))
                    ps1 = p1.enter_context(
                        tc.tile_pool(name=f"ps1{rep}", bufs=2, space="PSUM")
                    )

                    # view2 windows -> kT; view1 windows -> qT
                    for src, wgt, bias, dstT in (
                        (v2p, wk_s, bk_s, kT_s),
                        (v1p, wq_s, bq_s, qT_s),
                    ):
                        for j in range(nch):
                            jw = slice(j * nwin, (j + 1) * nwin)
                            xs = xst.tile([128, CC, nwin], f32, tag="xs")
                            nc.sync.dma_start(xs[:], src[:, :, jw])
                            xr = xrp.tile([128, CC, nwin], f32r, tag="xr")
                            nc.vector.tensor_copy(xr[:], xs[:])
                            ps = ps1.tile([64, nwin], f32, tag="psqk")
                            nc.tensor.matmul(
                                ps[:], bias[:], ones_row[:, :nwin], start=True, stop=False
                            )
                            for cc in range(CC):
                                nc.tensor.matmul(
                                    ps[:],
                                    wgt[:, cc, :],
                                    xr[:, cc, :],
                                    start=False,
                                    stop=(cc == CC - 1),
                                )
                            nc.scalar.activation(dstT[:, jw], ps[:], AF.Copy)

                    # view2 m-tile windows (re-streamed) -> vT (m partitions, C free)
                    for j in range(nch):
                        jw = slice(j * nwin, (j + 1) * nwin)
                        xs = xst.tile([128, CC, nwin], f32, tag="xs")
                        nc.sync.dma_start(xs[:], v2p[:, :, jw])
                        xr = xrp.tile([128, CC, nwin], f32r, tag="xr")
                        nc.vector.tensor_copy(xr[:], xs[:])
                        for mi in range(nwin // 128):
                            m = j * (nwin // 128) + mi
                            miw = slice(mi * 128, (mi + 1) * 128)
                            ps = ps1.tile([128, C], f32, tag="psv")
                            nc.tensor.matmul(
                                ps[:], ones_p[:], bv_s[:], start=True, stop=False
                            )
                            for cc in range(CC):
                                nc.tensor.matmul(
                                    ps[:],
                                    xr[:, cc, miw],
                                    wv_s[:, cc, :],
                                    start=False,
                                    stop=(cc == CC - 1),
                                )
                            nc.scalar.activation(vT_s[:, m, :], ps[:], AF.Copy)

                # ================= phase 2: attention =================
                with ExitStack() as p2:
                    psS = p2.enter_context(
                        tc.tile_pool(name=f"psS{rep}", bufs=2, space="PSUM")
                    )
                    psA = p2.enter_context(
                        tc.tile_pool(name=f"psA{rep}", bufs=1, space="PSUM")
                    )
                    psL = p2.enter_context(
                        tc.tile_pool(name=f"psL{rep}", bufs=1, space="PSUM")
                    )
                    expp = p2.enter_context(tc.tile_pool(name=f"expp{rep}", bufs=8))
                    smalls = p2.enter_context(tc.tile_pool(name=f"smalls{rep}", bufs=2))
                    rbp = p2.enter_context(tc.tile_pool(name=f"rbp{rep}", bufs=2))
                    resp = p2.enter_context(tc.tile_pool(name=f"resp{rep}", bufs=3))
                    outp_sb = p2.enter_context(tc.tile_pool(name=f"outp{rep}", bufs=3))

                    for j in range(nch):
                        jw = slice(j * nwin, (j + 1) * nwin)
                        # one PSUM tile (= one full bank) per output c-chunk:
                        # accumulation groups must not share a bank (start=True
                        # clears the whole bank's has_written bits)
                        accs = [
                            psA.tile([128, nwin], f32, tag=f"acc{ct}", name=f"acc{ct}")
                            for ct in range(CC)
                        ]
                        accl = psL.tile([1, nwin], f32, tag="accl")
                        for m in range(mt):
                            mw = slice(m * 128, (m + 1) * 128)
                            st = psS.tile([128, nwin], f32, tag="st")
                            nc.tensor.matmul(
                                st[:], kT_s[:, mw], qT_s[:, jw], start=True, stop=True
                            )
                            ex = expp.tile([128, nwin], f32r, tag="ex")
                            nc.scalar.activation(ex[:], st[:], AF.Exp)
                            for ct in range(CC):
                                nc.tensor.matmul(
                                    accs[ct][:],
                                    vT_s[:, m, ct * 128 : (ct + 1) * 128],
                                    ex[:],
                                    start=(m == 0),
                                    stop=(m == mt - 1),
                                )
                            nc.tensor.matmul(
                                accl[:],
                                ones_col[:],
                                ex[:],
                                start=(m == 0),
                                stop=(m == mt - 1),
                            )
                        # epilogue: y = acc * (gamma/l) + view1
                        l_sb = smalls.tile([1, nwin], f32, tag="l")
                        nc.vector.tensor_copy(l_sb[:], accl[:])
                        r_sb = smalls.tile([1, nwin], f32, tag="r")
                        nc.vector.reciprocal(r_sb[:], l_sb[:])
                        rg_sb = smalls.tile([1, nwin], f32r, tag="rg")
                        nc.scalar.activation(rg_sb[:], r_sb[:], AF.Copy, scale=gam_s[:])
                        rb_ps = psS.tile([128, nwin], f32, tag="st")
                        nc.tensor.matmul(rb_ps[:], ones_p[:], rg_sb[:], start=True, stop=True)
                        rb_sb = rbp.tile([128, nwin], f32, tag="rb")
                        nc.vector.tensor_copy(rb_sb[:], rb_ps[:])
                        for ct in range(CC):
                            v1c = resp.tile([128, nwin], f32, tag="v1c")
                            nc.sync.dma_start(v1c[:], v1p[:, ct, jw])
                            t_sb = outp_sb.tile([128, nwin], f32, tag="t")
                            nc.vector.tensor_mul(t_sb[:], accs[ct][:], rb_sb[:])
                            o_sb = outp_sb.tile([128, nwin], f32, tag="o")
                            nc.vector.tensor_add(o_sb[:], t_sb[:], v1c[:])
                            nc.sync.dma_start(outp[:, ct, jw], o_sb[:])

        if repeat == 1:
            emit_rep(0)
        else:
            with tc.For_i(0, repeat, 1):
                emit_rep(0)

    nc.compile()
    return nc


def _get_nc(n=N, repeat=1):
    key = (n, repeat)
    if key not in _compiled:
        _compiled[key] = _build(n=n, repeat=repeat)
    return _compiled[key]


def _run(nc, view1, view2, Wq, bq, Wk, bk, Wv, bv, gamma, n=N, **spmd_kwargs):
    from concourse.bass_utils import run_bass_kernel_spmd

    b = view1.shape[0]
    f = np.ascontiguousarray
    wqT = f(Wq.T.astype(np.float32))
    wkT = f(Wk.T.astype(np.float32))
    wvT = f(Wv.T.astype(np.float32))
    com = {
        "wqT": wqT,
        "wkT": wkT,
        "wvT": wvT,
        "bq": f(bq.reshape(1, D).astype(np.float32)),
        "bk": f(bk.reshape(1, D).astype(np.float32)),
        "bv": f(bv.reshape(1, C).astype(np.float32)),
        "gam": f(gamma.reshape(1, 1).astype(np.float32)),
    }
    in_maps = []
    for i in range(NCORES):
        bi = min(i, b - 1)  # replicate last sample if b < NCORES
        in_maps.append(
            {
                "v1": f(view1[bi].reshape(C, n).astype(np.float32)),
                "v2": f(view2[bi].reshape(C, n).astype(np.float32)),
                **com,
            }
        )
    res = run_bass_kernel_spmd(nc, in_maps, list(range(NCORES)), **spmd_kwargs)
    outs = [res.results[i]["out"] for i in range(b)]
    return np.stack(outs, axis=0)


def kernel(view1, view2, Wq, bq, Wk, bk, Wv, bv, gamma):
    view1 = np.asarray(view1)
    b, c, h, w = view1.shape
    n = h * w
    nc = _get_nc(n=n, repeat=1)
    out = _run(
        nc,
        np.asarray(view1),
        np.asarray(view2),
        np.asarray(Wq),
        np.asarray(bq),
        np.asarray(Wk),
        np.asarray(bk),
        np.asarray(Wv),
        np.asarray(bv),
        np.asarray(gamma),
        n=n,
    )
    return out.reshape(b, c, h, w).astype(np.float32)


# revision 13
# speedup vs baseline: 305.8904x; 305.8904x over previous
"""Cross-view attention Trainium2 kernel.

Reference computation (per sample b):
    q = Wq @ x1 + bq            (D=64, N)      x1 = view1[b] as (C, N)
    k = Wk @ x2 + bk            (D, N)
    v = Wv @ x2 + bv            (C, N)
    S = q^T k                   (N, N)
    P = softmax(S, axis=-1)
    out = v @ P^T               (C, N)
    y = gamma * out + x1

Sharding: data-parallel over batch B=8 across the 8 NeuronCores (one
sample per core), no collectives.

Device algorithm (per core):
  - Precision split: the logit chain (q/k projections, Q^T K) runs in
    float32r (~1.5e-4 matmul error, 2 PE-cycles/row) because exp
    amplifies logit error; the value chain (v projection, P.V, softmax
    denominator) runs in bf16 (1 cycle/row) where error stays relative.
  - Projections computed directly in the layouts attention needs:
    qT, kT as (D=64 partitions, N free), vT as (m partitions, C free).
  - Attention computed transposed: S^T tiles (m=128 partitions, n=512
    free) = kT_tile^T @ qT via K=64 matmuls packed two-at-a-time into
    disjoint PE row groups (tile_position); exp on ScalarE (no max
    subtraction: logits are bounded ~+-50, exp stays in fp32 range);
    P^T tiles feed out[c,n] += vT^T @ expS^T accumulated over all m in
    PSUM, and the softmax denominator l[n] comes from a ones-column
    matmul accumulated alongside.  Final: out = acc * (gamma/l) + view1.
"""

import sys

if "/opt/trn_rl_repo" not in sys.path:
    sys.path.insert(0, "/opt/trn_rl_repo")

import numpy as np

B, C, H, W = 8, 512, 64, 64
D = C // 8            # 64
N = H * W             # 4096
CC = C // 128         # 4 chunks of the channel dim
NCORES = 8

_compiled = {}


def _build(n=N, repeat=1, nwin=512):
    from contextlib import ExitStack

    import concourse.mybir as mybir
    import concourse.tile as tile
    from concourse import bacc

    dt = mybir.dt
    f32, f32r, bf16 = dt.float32, dt.float32r, dt.bfloat16
    AF = mybir.ActivationFunctionType

    nwin = min(nwin, n)
    nch = n // nwin       # output n-chunks
    mt = n // 128         # m tiles (key/value rows per tile)

    nc = bacc.Bacc("TRN2", target_bir_lowering=False, debug=False)
    v1 = nc.dram_tensor("v1", [C, n], f32, kind="ExternalInput").ap()
    v2 = nc.dram_tensor("v2", [C, n], f32, kind="ExternalInput").ap()
    wqT = nc.dram_tensor("wqT", [C, D], f32, kind="ExternalInput").ap()
    wkT = nc.dram_tensor("wkT", [C, D], f32, kind="ExternalInput").ap()
    wvT = nc.dram_tensor("wvT", [C, C], f32, kind="ExternalInput").ap()
    bq = nc.dram_tensor("bq", [1, D], f32, kind="ExternalInput").ap()
    bk = nc.dram_tensor("bk", [1, D], f32, kind="ExternalInput").ap()
    bv = nc.dram_tensor("bv", [1, C], f32, kind="ExternalInput").ap()
    gam = nc.dram_tensor("gam", [1, 1], f32, kind="ExternalInput").ap()
    out = nc.dram_tensor("out", [C, n], f32, kind="ExternalOutput").ap()

    v1p = v1.rearrange("(cc p) n -> p cc n", p=128)
    v2p = v2.rearrange("(cc p) n -> p cc n", p=128)
    outp = out.rearrange("(cc p) n -> p cc n", p=128)

    with tile.TileContext(nc) as tc, ExitStack() as top:
        consts = top.enter_context(tc.tile_pool(name="consts", bufs=1))

        # ---- constants ----
        wq_s = consts.tile([128, CC, D], f32r, tag="wq")
        wk_s = consts.tile([128, CC, D], f32r, tag="wk")
        wv_s = consts.tile([128, CC, C], bf16, tag="wv")
        bqc_s = consts.tile([D, 1], f32, tag="bqc")   # ACT bias column
        bkc_s = consts.tile([D, 1], f32, tag="bkc")
        bv_s = consts.tile([1, C], bf16, tag="bv")
        gam_s = consts.tile([1, 1], f32, tag="gam")
        ones_row = consts.tile([1, C], bf16, tag="ones_row")   # K=1 rhs (vT bias)
        ones_col = consts.tile([128, 1], bf16, tag="ones_col")  # K=128, M=1 lhsT (l)
        ones_p = consts.tile([1, 128], bf16, tag="ones_pb")  # K=1, M=128 lhsT (vT bias)
        ones_pr = consts.tile([1, 128], f32r, tag="ones_pr")  # K=1, M=128 lhsT (rb bcast)

        with ExitStack() as p0:
            wstp = p0.enter_context(tc.tile_pool(name="wst", bufs=1))
            stage_w = wstp.tile([128, CC, C], f32, tag="stage_w")
            nc.sync.dma_start(stage_w[:, :, :D], wqT.rearrange("(cc p) d -> p cc d", p=128))
            nc.vector.tensor_copy(wq_s[:], stage_w[:, :, :D])
            nc.sync.dma_start(stage_w[:, :, D : 2 * D], wkT.rearrange("(cc p) d -> p cc d", p=128))
            nc.vector.tensor_copy(wk_s[:], stage_w[:, :, D : 2 * D])
            nc.sync.dma_start(stage_w[:], wvT.rearrange("(cc p) c -> p cc c", p=128))
            nc.vector.tensor_copy(wv_s[:], stage_w[:])

            nc.sync.dma_start(bqc_s[:], bq.rearrange("o d -> d o"))
            nc.sync.dma_start(bkc_s[:], bk.rearrange("o d -> d o"))
            nc.sync.dma_start(gam_s[:], gam[:])
            stage_b = wstp.tile([1, C], f32, tag="stage_b")
            nc.sync.dma_start(stage_b[:], bv[:])
            nc.vector.tensor_copy(bv_s[:], stage_b[:])

            ones_f32 = wstp.tile([128, C], f32, tag="ones_f32")
            nc.vector.memset(ones_f32[:], 1.0)
            nc.vector.tensor_copy(ones_row[:], ones_f32[:1, :])
            nc.vector.tensor_copy(ones_col[:], ones_f32[:, :1])
            nc.vector.tensor_copy(ones_p[:], ones_f32[:1, :128])
            nc.vector.tensor_copy(ones_pr[:], ones_f32[:1, :128])

        def emit_rep(rep):
            with ExitStack() as rctx:
                per = rctx.enter_context(tc.tile_pool(name=f"persist{rep}", bufs=1))
                # qT/kT duplicated across both partition halves for the
                # row-packed (tile_position) S^T matmuls
                qT_s = per.tile([128, n], f32r, tag="qT")
                kT_s = per.tile([128, n], f32r, tag="kT")
                vT_s = per.tile([128, mt, C], bf16, tag="vT")

                # ================= phase 1: projections =================
                with ExitStack() as p1:
                    xst = p1.enter_context(tc.tile_pool(name=f"xst{rep}", bufs=3))
                    xrp = p1.enter_context(tc.tile_pool(name=f"xrp{rep}", bufs=3))
                    ps1 = p1.enter_context(
                        tc.tile_pool(name=f"ps1{rep}", bufs=2, space="PSUM")
                    )

                    # view2 windows -> kT; view1 windows -> qT   (f32r chain)
                    for src, wgt, biasc, dstT in (
                        (v2p, wk_s, bkc_s, kT_s),
                        (v1p, wq_s, bqc_s, qT_s),
                    ):
                        for j in range(nch):
                            jw = slice(j * nwin, (j + 1) * nwin)
                            xs = xst.tile([128, CC, nwin], f32, tag="xs")
                            nc.sync.dma_start(xs[:], src[:, :, jw])
                            xr = xrp.tile([128, CC, nwin], f32r, tag="xr")
                            nc.vector.tensor_copy(xr[:], xs[:])
                            ps = ps1.tile([64, nwin], f32, tag="psqk")
                            for cc in range(CC):
                                nc.tensor.matmul(
                                    ps[:],
                                    wgt[:, cc, :],
                                    xr[:, cc, :],
                                    start=(cc == 0),
                                    stop=(cc == CC - 1),
                                )
                            nc.scalar.activation(
                                dstT[:64, jw], ps[:], AF.Identity, bias=biasc[:]
                            )
                            # duplicate into the upper partition half
                            nc.sync.dma_start(dstT[64:128, jw], dstT[:64, jw])

                    # view2 m-tile windows (re-streamed) -> vT   (bf16 chain)
                    for j in range(nch):
                        jw = slice(j * nwin, (j + 1) * nwin)
                        xs = xst.tile([128, CC, nwin], f32, tag="xs")
                        nc.sync.dma_start(xs[:], v2p[:, :, jw])
                        xb = xrp.tile([128, CC, nwin], bf16, tag="xb")
                        nc.vector.tensor_copy(xb[:], xs[:])
                        for mi in range(nwin // 128):
                            m = j * (nwin // 128) + mi
                            miw = slice(mi * 128, (mi + 1) * 128)
                            ps = ps1.tile([128, C], f32, tag="psv")
                            nc.tensor.matmul(
                                ps[:], ones_p[:], bv_s[:], start=True, stop=False
                            )
                            for cc in range(CC):
                                nc.tensor.matmul(
                                    ps[:],
                                    xb[:, cc, miw],
                                    wv_s[:, cc, :],
                                    start=False,
                                    stop=(cc == CC - 1),
                                )
                            nc.scalar.activation(vT_s[:, m, :], ps[:], AF.Copy)

                # ================= phase 2: attention =================
                with ExitStack() as p2:
                    psS = p2.enter_context(
                        tc.tile_pool(name=f"psS{rep}", bufs=3, space="PSUM")
                    )
                    psA = p2.enter_context(
                        tc.tile_pool(name=f"psA{rep}", bufs=1, space="PSUM")
                    )
                    psL = p2.enter_context(
                        tc.tile_pool(name=f"psL{rep}", bufs=1, space="PSUM")
                    )
                    expp = p2.enter_context(tc.tile_pool(name=f"expp{rep}", bufs=10))
                    smalls = p2.enter_context(tc.tile_pool(name=f"smalls{rep}", bufs=2))
                    rbp = p2.enter_context(tc.tile_pool(name=f"rbp{rep}", bufs=2))
                    resp = p2.enter_context(tc.tile_pool(name=f"resp{rep}", bufs=3))
                    outp_sb = p2.enter_context(tc.tile_pool(name=f"outp{rep}", bufs=3))

                    for j in range(nch):
                        jw = slice(j * nwin, (j + 1) * nwin)
                        # one PSUM tile (= one full bank) per output c-chunk:
                        # accumulation groups must not share a bank (start=True
                        # clears the whole bank's has_written bits)
                        accs = [
                            psA.tile([128, nwin], f32, tag=f"acc{ct}", name=f"acc{ct}")
                            for ct in range(CC)
                        ]
                        accl = psL.tile([1, nwin], f32, tag="accl")
                        for m2 in range(mt // 2):
                            sts = []
                            exs = []
                            for half in (0, 1):
                                m = 2 * m2 + half
                                mw = slice(m * 128, (m + 1) * 128)
                                hp = slice(64 * half, 64 * half + 64)
                                st = psS.tile([128, nwin], f32, tag="st", name="st")
                                nc.tensor.matmul(
                                    st[:],
                                    kT_s[hp, mw],
                                    qT_s[hp, jw],
                                    start=True,
                                    stop=True,
                                    tile_position=(64 * half, 0),
                                )
                                sts.append(st)
                            for half in (0, 1):
                                ex = expp.tile([128, nwin], bf16, tag="ex", name="ex")
                                nc.scalar.activation(ex[:], sts[half][:], AF.Exp)
                                exs.append(ex)
                            for half in (0, 1):
                                m = 2 * m2 + half
                                ex = exs[half]
                                for ct in range(CC):
                                    nc.tensor.matmul(
                                        accs[ct][:],
                                        vT_s[:, m, ct * 128 : (ct + 1) * 128],
                                        ex[:],
                                        start=(m == 0),
                                        stop=(m == mt - 1),
                                    )
                                nc.tensor.matmul(
                                    accl[:],
                                    ones_col[:],
                                    ex[:],
                                    start=(m == 0),
                                    stop=(m == mt - 1),
                                )
                        # epilogue: y = acc * (gamma/l) + view1
                        l_sb = smalls.tile([1, nwin], f32, tag="l")
                        nc.vector.tensor_copy(l_sb[:], accl[:])
                        r_sb = smalls.tile([1, nwin], f32, tag="r")
                        nc.vector.reciprocal(r_sb[:], l_sb[:])
                        rg_sb = smalls.tile([1, nwin], f32r, tag="rg")
                        nc.scalar.activation(rg_sb[:], r_sb[:], AF.Copy, scale=gam_s[:])
                        rb_ps = psS.tile([128, nwin], f32, tag="st", name="rb_ps")
                        nc.tensor.matmul(rb_ps[:], ones_pr[:], rg_sb[:], start=True, stop=True)
                        rb_sb = rbp.tile([128, nwin], f32, tag="rb")
                        nc.vector.tensor_copy(rb_sb[:], rb_ps[:])
                        for ct in range(CC):
                            v1c = resp.tile([128, nwin], f32, tag="v1c")
                            nc.sync.dma_start(v1c[:], v1p[:, ct, jw])
                            t_sb = outp_sb.tile([128, nwin], f32, tag="t")
                            nc.vector.tensor_mul(t_sb[:], accs[ct][:], rb_sb[:])
                            o_sb = outp_sb.tile([128, nwin], f32, tag="o")
                            nc.vector.tensor_add(o_sb[:], t_sb[:], v1c[:])
                            nc.sync.dma_start(outp[:, ct, jw], o_sb[:])

        if repeat == 1:
            emit_rep(0)
        else:
            with tc.For_i(0, repeat, 1):
                emit_rep(0)

    nc.compile()
    return nc


def _get_nc(n=N, repeat=1):
    key = (n, repeat)
    if key not in _compiled:
        _compiled[key] = _build(n=n, repeat=repeat)
    return _compiled[key]


def _run(nc, view1, view2, Wq, bq, Wk, bk, Wv, bv, gamma, n=N, **spmd_kwargs):
    from concourse.bass_utils import run_bass_kernel_spmd

    b = view1.shape[0]
    f = np.ascontiguousarray
    com = {
        "wqT": f(Wq.T.astype(np.float32)),
        "wkT": f(Wk.T.astype(np.float32)),
        "wvT": f(Wv.T.astype(np.float32)),
        "bq": f(bq.reshape(1, D).astype(np.float32)),
        "bk": f(bk.reshape(1, D).astype(np.float32)),
        "bv": f(bv.reshape(1, C).astype(np.float32)),
        "gam": f(gamma.reshape(1, 1).astype(np.float32)),
    }
    in_maps = []
    for i in range(NCORES):
        bi = min(i, b - 1)  # replicate last sample if b < NCORES
        in_maps.append(
            {
                "v1": f(view1[bi].reshape(C, n).astype(np.float32)),
                "v2": f(view2[bi].reshape(C, n).astype(np.float32)),
                **com,
            }
        )
    res = run_bass_kernel_spmd(nc, in_maps, list(range(NCORES)), **spmd_kwargs)
    outs = [res.results[i]["out"] for i in range(b)]
    return np.stack(outs, axis=0)


def kernel(view1, view2, Wq, bq, Wk, bk, Wv, bv, gamma):
    view1 = np.asarray(view1)
    b, c, h, w = view1.shape
    n = h * w
    nc = _get_nc(n=n, repeat=1)
    out = _run(
        nc,
        np.asarray(view1),
        np.asarray(view2),
        np.asarray(Wq),
        np.asarray(bq),
        np.asarray(Wk),
        np.asarray(bk),
        np.asarray(Wv),
        np.asarray(bv),
        np.asarray(gamma),
        n=n,
    )
    return out.reshape(b, c, h, w).astype(np.float32)
